# revision 19
# baseline (speedup 1.0000x reference)
"""DeepseekV4 indexer kernel for 8 trn2 NeuronCores (Bass/Tile).

Three launches, all GEMMs in exact fp32 with N=512 moving dims
(fp32 LOW_HIGH 2-pass runs ~2.2 cyc/row at N=512 vs 4 cyc/row at N=256):

  LA (per core): K-sharded fused kv/score/wts GEMM partial (own 896 HID
     rows x all 2048 tokens, transposed output) + head-sharded q GEMM
     (own 8 heads x all tokens, output [D, T] per head) + RoPE applied
     via a half-swap permutation matmul.  Host reduces the kv partials
     and reshuffles q to token shards.
  LB (per core): the compressor (softmax window -> RMSNorm -> RoPE) on
     its 256-token shard of the reduced kv_score, plus |wts|/sign prep.
     Host gathers compressed K.
  LC (per core): qk matmuls against full K, relu*wts head accumulation
     (Scalar engine: relu(qk*|w|); GpSimd: sign-multiply-accumulate;
     Vector engine reserved for the iterated top-8 extraction), causal
     mask and top-256 in descending-score order.

kernel(**inputs) takes FULL unsharded inputs, returns [2048,256] int32.
"""
import sys
sys.path.insert(0, '/opt/trn_rl_repo')

from contextlib import ExitStack

import numpy as np

import concourse.bass as bass
import concourse.bacc as bacc
import concourse.tile as tile
from concourse import mybir
from concourse.bass_utils import run_bass_kernel_spmd
from concourse.masks import make_identity

T, HID, QR_DIM, H, D, TOPK, R = 2048, 7168, 1536, 64, 128, 256, 4
C = T // R
NC = 8
KSH = HID // NC          # 896 contraction rows per core in LA
HSH = H // NC            # 8 heads per core in LA
EPS = 1e-6
F32 = mybir.dt.float32
F16 = mybir.dt.float16
I32 = mybir.dt.int32
U32 = mybir.dt.uint32
WTS_SCALE = float(H ** -0.5) * float(D ** -0.5)
NEG = -1e30
WIDTHS = (256, 512)      # candidate widths for (low tile j<=7, high tile)

_cache = {}


# --------------------------------------------------------------------------
# LA: kv partial GEMM (K-sharded) + q GEMM (head-sharded) + rope
# --------------------------------------------------------------------------
def _build_la():
    nc = bacc.Bacc()
    hidt_h = nc.declare_dram_parameter("hidt_h", [KSH, T], F16, isOutput=False)
    hidt_l = nc.declare_dram_parameter("hidt_l", [KSH, T], F16, isOutput=False)
    wcomb_h = nc.declare_dram_parameter("wcomb_h", [KSH, 576], F16,
                                        isOutput=False)
    wcomb_l = nc.declare_dram_parameter("wcomb_l", [KSH, 576], F16,
                                        isOutput=False)
    qrt_h = nc.declare_dram_parameter("qrt_h", [QR_DIM, T], F16,
                                      isOutput=False)
    qrt_l = nc.declare_dram_parameter("qrt_l", [QR_DIM, T], F16,
                                      isOutput=False)
    wq_h = nc.declare_dram_parameter("wq_h", [HSH, 128, 12, 128], F16,
                                     isOutput=False)
    wq_l = nc.declare_dram_parameter("wq_l", [HSH, 128, 12, 128], F16,
                                     isOutput=False)
    cc = nc.declare_dram_parameter("cc", [128, T], F32, isOutput=False)
    ss = nc.declare_dram_parameter("ss", [128, T], F32, isOutput=False)
    perm = nc.declare_dram_parameter("perm", [128, 128], F32, isOutput=False)
    kv_part = nc.declare_dram_parameter("kv_part", [576, T], F32,
                                        isOutput=True)
    qro = nc.declare_dram_parameter("qro", [HSH, 128, T], F32, isOutput=True)

    with tile.TileContext(nc) as tc, ExitStack() as ctx:
        const = ctx.enter_context(tc.tile_pool(name="const", bufs=1))
        stage = ctx.enter_context(tc.tile_pool(name="stage", bufs=2))
        qrp = ctx.enter_context(tc.tile_pool(name="qrp", bufs=2))

        perm_sb = const.tile([128, 128], F32)
        nc.sync.dma_start(out=perm_sb, in_=perm[:])
        cc_sb = const.tile([128, T], F32)
        nc.sync.dma_start(out=cc_sb, in_=cc[:])
        ss_sb = const.tile([128, T], F32)
        nc.sync.dma_start(out=ss_sb, in_=ss[:])

        # all 8 heads' Wq hi/lo tiles resident: [128, 12, 128] f16 (3KB/part)
        wqt = []
        for m in range(HSH):
            th = const.tile([128, 12, 128], F16, tag=f"wqh{m}", name=f"wqh{m}")
            nc.sync.dma_start(out=th, in_=wq_h[m])
            tl = const.tile([128, 12, 128], F16, tag=f"wql{m}", name=f"wql{m}")
            nc.sync.dma_start(out=tl, in_=wq_l[m])
            wqt.append((th, tl))

        # ---- kv/score/wts partial GEMM: out [576, 2048] (transposed) ----
        with tc.tile_pool(name="wcp", bufs=1) as wcp, \
             tc.tile_pool(name="hidp", bufs=14) as hidp, \
             tc.tile_pool(name="kvps", bufs=2, space="PSUM") as kvps, \
             tc.tile_pool(name="kvo", bufs=3) as kvo:
            wct = []
            for kc in range(7):
                th = wcp.tile([128, 576], F16, tag=f"wch{kc}", name=f"wch{kc}")
                nc.sync.dma_start(out=th,
                                  in_=wcomb_h[kc * 128:(kc + 1) * 128, :])
                tl = wcp.tile([128, 576], F16, tag=f"wcl{kc}", name=f"wcl{kc}")
                nc.sync.dma_start(out=tl,
                                  in_=wcomb_l[kc * 128:(kc + 1) * 128, :])
                wct.append((th, tl))
            for b in range(4):
                hbt = []
                for kc in range(7):
                    ht = hidp.tile([128, 512], F16, tag="hidh")
                    nc.sync.dma_start(
                        out=ht,
                        in_=hidt_h[kc * 128:(kc + 1) * 128,
                                   b * 512:(b + 1) * 512])
                    hl = hidp.tile([128, 512], F16, tag="hidl")
                    nc.sync.dma_start(
                        out=hl,
                        in_=hidt_l[kc * 128:(kc + 1) * 128,
                                   b * 512:(b + 1) * 512])
                    hbt.append((ht, hl))
                for m in range(5):
                    M = 128 if m < 4 else 64
                    ms = slice(m * 128, m * 128 + M)
                    p1 = kvps.tile([128, 512], F32, tag="kv1")
                    p2 = kvps.tile([128, 512], F32, tag="kv2")
                    for kc in range(7):
                        nc.tensor.matmul(p1[:M, :], wct[kc][0][:, ms],
                                         hbt[kc][0], start=(kc == 0),
                                         stop=(kc == 6))
                    for kc in range(7):
                        nc.tensor.matmul(p2[:M, :], wct[kc][0][:, ms],
                                         hbt[kc][1], start=(kc == 0),
                                         stop=False)
                        nc.tensor.matmul(p2[:M, :], wct[kc][1][:, ms],
                                         hbt[kc][0], start=False,
                                         stop=(kc == 6))
                    ob = kvo.tile([128, 512], F32, tag="kvo")
                    nc.scalar.copy(ob[:M, :], p1[:M, :])
                    nc.vector.scalar_tensor_tensor(
                        out=ob[:M, :], in0=p2[:M, :], scalar=1.0 / 2048.0,
                        in1=ob[:M, :], op0=mybir.AluOpType.mult,
                        op1=mybir.AluOpType.add)
                    nc.sync.dma_start(
                        out=kv_part[m * 128:m * 128 + M,
                                    b * 512:(b + 1) * 512],
                        in_=ob[:M, :])

        # ---- q GEMM (8 heads) + rope, quarter-streamed over tokens ----
        qps = ctx.enter_context(tc.tile_pool(name="qps", bufs=2,
                                             space="PSUM"))
        swps = ctx.enter_context(tc.tile_pool(name="swps", bufs=2,
                                              space="PSUM"))
        for qtr in range(4):
            qth = qrp.tile([128, 12, 512], F16, tag="qrh")
            nc.sync.dma_start(
                out=qth,
                in_=qrt_h[:].rearrange("(kc p) t -> p kc t", p=128)[
                    :, :, qtr * 512:(qtr + 1) * 512])
            qtl = qrp.tile([128, 12, 512], F16, tag="qrl")
            nc.sync.dma_start(
                out=qtl,
                in_=qrt_l[:].rearrange("(kc p) t -> p kc t", p=128)[
                    :, :, qtr * 512:(qtr + 1) * 512])
            for m in range(HSH):
                p1 = qps.tile([128, 512], F32, tag="q1")
                p2 = qps.tile([128, 512], F32, tag="q2")
                for kc in range(12):
                    nc.tensor.matmul(p1, wqt[m][0][:, kc, :], qth[:, kc, :],
                                     start=(kc == 0), stop=(kc == 11))
                for kc in range(12):
                    nc.tensor.matmul(p2, wqt[m][0][:, kc, :], qtl[:, kc, :],
                                     start=(kc == 0), stop=False)
                    nc.tensor.matmul(p2, wqt[m][1][:, kc, :], qth[:, kc, :],
                                     start=False, stop=(kc == 11))
                q_sb = stage.tile([128, 512], F32, tag="qsb")
                nc.scalar.copy(q_sb, p1)
                nc.vector.scalar_tensor_tensor(
                    out=q_sb, in0=p2, scalar=1.0 / 2048.0, in1=q_sb,
                    op0=mybir.AluOpType.mult, op1=mybir.AluOpType.add)
                ps_sw = swps.tile([128, 512], F32, tag="sw")
                nc.tensor.matmul(ps_sw, perm_sb, q_sb, start=True, stop=True)
                o = qtr * 512
                t1 = stage.tile([128, 512], F32, tag="t1")
                nc.vector.tensor_mul(t1, q_sb, cc_sb[:, o:o + 512])
                t2 = stage.tile([128, 512], F32, tag="t2")
                nc.vector.tensor_mul(t2, ps_sw, ss_sb[:, o:o + 512])
                qo = stage.tile([128, 512], F32, tag="qo")
                nc.gpsimd.tensor_add(qo, t1, t2)
                nc.sync.dma_start(out=qro[m][:, o:o + 512], in_=qo)

    nc.finalize()
    return nc


# --------------------------------------------------------------------------
# LB: compressor on own 256-token shard of reduced kv_score + wts prep
# --------------------------------------------------------------------------
def _build_lb():
    nc = bacc.Bacc()
    kvst = nc.declare_dram_parameter("kvst", [576, 264], F32, isOutput=False)
    ape = nc.declare_dram_parameter("ape", [8, D], F32, isOutput=False)
    rmsw = nc.declare_dram_parameter("rmsw", [D], F32, isOutput=False)
    cs_k = nc.declare_dram_parameter("cs_k", [64, D], F32, isOutput=False)
    haloflag = nc.declare_dram_parameter("haloflag", [2], F32, isOutput=False)
    k_loc = nc.declare_dram_parameter("k_loc", [64, D], F32, isOutput=True)
    wabs = nc.declare_dram_parameter("wabs", [256, H], F32, isOutput=True)
    wsgn = nc.declare_dram_parameter("wsgn", [256, H], F32, isOutput=True)

    with tile.TileContext(nc) as tc, ExitStack() as ctx:
        const = ctx.enter_context(tc.tile_pool(name="const", bufs=1))
        work = ctx.enter_context(tc.tile_pool(name="work", bufs=2))

        ident = const.tile([128, 128], F32)
        make_identity(nc, ident)

        def tp(ps_out, in_sb):
            p = in_sb.shape[0]
            nc.tensor.transpose(ps_out, in_sb, ident[:p, :p])

        # kv/score tiles straight from DRAM (rows of kvst)
        kvt = []
        for r4 in range(4):
            t = const.tile([128, 264], F32, tag=f"kv{r4}", name=f"kv{r4}")
            nc.sync.dma_start(out=t, in_=kvst[r4 * 128:(r4 + 1) * 128, :])
            kvt.append(t)
        kv_old, kv_new, sc_old, sc_new = kvt
        wts_sb = const.tile([64, 264], F32)
        nc.sync.dma_start(out=wts_sb, in_=kvst[512:576, :])

        with tc.tile_pool(name="tpsB", bufs=2, space="PSUM") as tpsB:
            # wts -> [t, h]; emit |w|*scale and sign
            for s in range(2):
                ps = tpsB.tile([128, 64], F32, tag="wtp")
                tp(ps, wts_sb[:, 4 + 132 * s:132 + 132 * s])
                wa = work.tile([128, 64], F32, tag="wa")
                nc.scalar.activation(wa, ps, mybir.ActivationFunctionType.Abs,
                                     scale=WTS_SCALE)
                nc.sync.dma_start(out=wabs[128 * s:128 * (s + 1), :], in_=wa)
                sg = work.tile([128, 64], F32, tag="sg")
                nc.scalar.activation(sg, ps,
                                     mybir.ActivationFunctionType.Sign)
                nc.sync.dma_start(out=wsgn[128 * s:128 * (s + 1), :], in_=sg)

            # ape transposed + replicated [128, 32, 8]
            ape_st = work.tile([8, D], F32, tag="ape_st")
            nc.sync.dma_start(out=ape_st, in_=ape[:])
            aps = tpsB.tile([128, 8], F32, tag="apetp")
            tp(aps, ape_st)
            apeT = const.tile([128, 8], F32)
            nc.scalar.copy(apeT, aps)
            ape_rep = const.tile([128, 32, 8], F32)
            for g in range(32):
                nc.vector.tensor_copy(ape_rep[:, g, :], apeT)

            rms_rep = const.tile([32, D], F32)
            nc.sync.dma_start(out=rms_rep, in_=bass.AP(
                tensor=rmsw, offset=0, ap=[[0, 32], [1, D]]))

            cs_st = []
            for s in range(2):
                cst = const.tile([32, D], F32, tag=f"cs{s}", name=f"cs{s}")
                nc.sync.dma_start(out=cst, in_=cs_k[32 * s:32 * s + 32, :])
                cs_st.append(cst)

            hf = []
            for s in range(2):
                h = const.tile([128, 1], F32, tag=f"hf{s}")
                nc.sync.dma_start(out=h, in_=bass.AP(
                    tensor=haloflag, offset=s, ap=[[0, 128], [1, 1]]))
                hf.append(h)

            for s in range(2):
                o = 132 * s
                gates = work.tile([128, 32, 8], F32, tag="gates")
                so_v = sc_old[:, o:o + 128].rearrange("p (g x) -> p g x", x=4)
                sn_v = sc_new[:, o + 4:o + 132].rearrange(
                    "p (g x) -> p g x", x=4)
                ko_v = kv_old[:, o:o + 128].rearrange("p (g x) -> p g x", x=4)
                kn_v = kv_new[:, o + 4:o + 132].rearrange(
                    "p (g x) -> p g x", x=4)
                nc.vector.tensor_add(gates[:, :, 0:4], so_v,
                                     ape_rep[:, :, 0:4])
                nc.vector.tensor_add(gates[:, :, 4:8], sn_v,
                                     ape_rep[:, :, 4:8])
                nc.vector.tensor_scalar(gates[:, 0, 0:4], gates[:, 0, 0:4],
                                        hf[s], None, op0=mybir.AluOpType.add)
                gmax = work.tile([128, 32], F32, tag="gmax")
                nc.vector.reduce_max(gmax, gates, axis=mybir.AxisListType.X)
                nc.vector.tensor_sub(gates, gates,
                                     gmax.to_broadcast([128, 32, 8]))
                ex = work.tile([128, 32, 8], F32, tag="ex")
                nc.scalar.activation(ex, gates,
                                     mybir.ActivationFunctionType.Exp)
                den = work.tile([128, 32], F32, tag="den")
                nc.vector.reduce_sum(den, ex, axis=mybir.AxisListType.X)
                rec = work.tile([128, 32], F32, tag="rec")
                nc.vector.reciprocal(rec, den)
                w8 = work.tile([128, 32, 8], F32, tag="w8")
                nc.vector.tensor_mul(w8, ex, rec.to_broadcast([128, 32, 8]))
                prod = work.tile([128, 32, 8], F32, tag="prod")
                nc.vector.tensor_mul(prod[:, :, 0:4], w8[:, :, 0:4], ko_v)
                nc.vector.tensor_mul(prod[:, :, 4:8], w8[:, :, 4:8], kn_v)
                comp = work.tile([128, 32], F32, tag="comp")
                nc.vector.reduce_sum(comp, prod, axis=mybir.AxisListType.X)

                cps = tpsB.tile([32, 128], F32, tag="ctp")
                tp(cps, comp)
                compT = work.tile([32, D], F32, tag="compT")
                nc.scalar.copy(compT, cps)

                sq = work.tile([32, D], F32, tag="sq")
                nc.vector.tensor_mul(sq, compT, compT)
                ssum = work.tile([32, 1], F32, tag="ssum")
                nc.vector.reduce_sum(ssum, sq, axis=mybir.AxisListType.X)
                nc.vector.tensor_scalar(ssum, ssum, 1.0 / D, EPS,
                                        op0=mybir.AluOpType.mult,
                                        op1=mybir.AluOpType.add)
                rt = work.tile([32, 1], F32, tag="rt")
                nc.scalar.sqrt(rt, ssum)
                rs = work.tile([32, 1], F32, tag="rs")
                nc.vector.reciprocal(rs, rt)
                nc.vector.tensor_scalar(compT, compT, rs, None,
                                        op0=mybir.AluOpType.mult)
                nc.vector.tensor_mul(compT, compT, rms_rep)

                co = cs_st[s][:, 0:64]
                si = cs_st[s][:, 64:128]
                x1 = compT[:, 0:64]
                x2 = compT[:, 64:128]
                tmp = work.tile([32, D], F32, tag="ktmp")
                kx = work.tile([32, D], F32, tag="kx")
                nc.vector.tensor_mul(kx[:, 0:64], x1, co)
                nc.vector.tensor_mul(tmp[:, 0:64], x2, si)
                nc.vector.tensor_sub(kx[:, 0:64], kx[:, 0:64], tmp[:, 0:64])
                nc.vector.tensor_mul(kx[:, 64:128], x2, co)
                nc.vector.tensor_mul(tmp[:, 64:128], x1, si)
                nc.vector.tensor_add(kx[:, 64:128], kx[:, 64:128],
                                     tmp[:, 64:128])
                nc.sync.dma_start(out=k_loc[32 * s:32 * s + 32, :], in_=kx)

    nc.finalize()
    return nc


# --------------------------------------------------------------------------
# LC: qk + relu*wts head-sum (ACT+GpSimd) + causal mask + top-k (DVE)
# --------------------------------------------------------------------------
def _build_lc():
    nc = bacc.Bacc()
    qro = nc.declare_dram_parameter("qro", [H, 128, 256], F32, isOutput=False)
    kt = nc.declare_dram_parameter("kt", [128, C], F32, isOutput=False)
    wabs = nc.declare_dram_parameter("wabs", [256, H], F32, isOutput=False)
    wsgn = nc.declare_dram_parameter("wsgn", [256, H], F32, isOutput=False)
    posm3 = nc.declare_dram_parameter("posm3", [256], F32, isOutput=False)
    out_idx = nc.declare_dram_parameter("out_idx", [256, TOPK], I32,
                                        isOutput=True)

    with tile.TileContext(nc) as tc, ExitStack() as ctx:
        const = ctx.enter_context(tc.tile_pool(name="const", bufs=1))
        work = ctx.enter_context(tc.tile_pool(name="work", bufs=2))
        tk = ctx.enter_context(tc.tile_pool(name="tk", bufs=2))
        yp = ctx.enter_context(tc.tile_pool(name="yp", bufs=3))
        qkps = ctx.enter_context(tc.tile_pool(name="qkps", bufs=3,
                                              space="PSUM"))

        kt_sb = const.tile([128, C], F32)
        nc.sync.dma_start(out=kt_sb, in_=kt[:])
        qrop = ctx.enter_context(tc.tile_pool(name="qrop", bufs=8))

        wabs_sb, wsgn_sb, pos_sb = [], [], []
        for tt in range(2):
            wa = const.tile([128, H], F32, tag=f"wa{tt}", name=f"wa{tt}")
            nc.sync.dma_start(out=wa, in_=wabs[tt * 128:(tt + 1) * 128, :])
            wabs_sb.append(wa)
            sg = const.tile([128, H], F32, tag=f"sg{tt}", name=f"sg{tt}")
            nc.sync.dma_start(out=sg, in_=wsgn[tt * 128:(tt + 1) * 128, :])
            wsgn_sb.append(sg)
            p = const.tile([128, 1], F32, tag=f"pos{tt}", name=f"pos{tt}")
            nc.sync.dma_start(out=p, in_=posm3[tt * 128:(tt + 1) * 128])
            pos_sb.append(p)

        c4p = const.tile([128, C], F32)
        nc.gpsimd.iota(c4p, pattern=[[4, C]], base=0, channel_multiplier=0,
                       allow_small_or_imprecise_dtypes=True)
        c4f = const.tile([128, C], F32)
        nc.vector.tensor_scalar(c4f, c4p, -1.0, None,
                                op0=mybir.AluOpType.mult)
        negs = const.tile([128, C], F32)
        nc.vector.memset(negs, NEG)
        neg1 = const.tile([128, TOPK], I32)
        nc.vector.memset(neg1, -1)

        acc = [const.tile([128, WIDTHS[tt]], F32, tag=f"acc{tt}",
                          name=f"acc{tt}") for tt in range(2)]

        def emit_topk(tt):
            W = WIDTHS[tt]
            cmp = work.tile([128, C], F32, tag="cmp", name="cmp")
            nc.vector.tensor_scalar(cmp[:, :W], c4f[:, :W], pos_sb[tt], None,
                                    op0=mybir.AluOpType.add)
            mbit = work.tile([128, C], U32, tag="mbit", name="mbit")
            nc.vector.tensor_scalar(mbit[:, :W], cmp[:, :W], 0.0, None,
                                    op0=mybir.AluOpType.is_lt)
            nc.vector.copy_predicated(acc[tt], mbit[:, :W], negs[:, :W])

            idx = tk.tile([128, TOPK], U32, tag="idx", name="idx")
            vals = acc[tt]
            for it in range(32):
                mx = tk.tile([128, 8], F32, tag="mx", name="mx")
                nc.vector.max(out=mx, in_=vals)
                nc.vector.max_index(out=idx[:, it * 8:(it + 1) * 8],
                                    in_max=mx, in_values=vals)
                nc.vector.match_replace(out=vals, in_to_replace=mx,
                                        in_values=vals, imm_value=NEG)

            idx32 = tk.tile([128, TOPK], I32, tag="idx32", name="idx32")
            nc.vector.tensor_copy(idx32, idx)
            rmp = work.tile([128, TOPK], F32, tag="rmp", name="rmp")
            nc.vector.tensor_scalar(rmp, c4f[:, :TOPK], pos_sb[tt], None,
                                    op0=mybir.AluOpType.add)
            rbit = work.tile([128, TOPK], U32, tag="rbit", name="rbit")
            nc.vector.tensor_scalar(rbit, rmp, 0.0, None,
                                    op0=mybir.AluOpType.is_lt)
            nc.vector.copy_predicated(idx32, rbit, neg1)
            nc.sync.dma_start(out=out_idx[tt * 128:(tt + 1) * 128, :],
                              in_=idx32)

        nc.vector.memset(acc[0], 0.0)
        nc.vector.memset(acc[1], 0.0)
        for h in range(H):
            qt = qrop.tile([128, 256], F32, tag="qro")
            nc.sync.dma_start(out=qt, in_=qro[h])
            for tt in range(2):
                W = WIDTHS[tt]
                ps_qk = qkps.tile([128, 512], F32, tag=f"qk{tt}",
                                  name=f"ps_qk{tt}")
                nc.tensor.matmul(ps_qk[:, :W],
                                 qt[:, tt * 128:(tt + 1) * 128],
                                 kt_sb[:, :W], start=True, stop=True)
                y = yp.tile([128, 512], F32, tag=f"y{tt}")
                nc.scalar.activation(y[:, :W], ps_qk[:, :W],
                                     mybir.ActivationFunctionType.Relu,
                                     scale=wabs_sb[tt][:, h:h + 1])
                nc.vector.scalar_tensor_tensor(
                    out=acc[tt], in0=y[:, :W], scalar=wsgn_sb[tt][:, h:h + 1],
                    in1=acc[tt], op0=mybir.AluOpType.mult,
                    op1=mybir.AluOpType.add)
        emit_topk(0)
        emit_topk(1)

    nc.finalize()
    return nc


def _get(name):
    if name not in _cache:
        _cache[name] = {"la": _build_la, "lb": _build_lb,
                        "lc": _build_lc}[name]()
    return _cache[name]


def kernel(hidden_states, qr, positions, W_fused, Wq, Wproj, ape, rms_weight,
           cos_sin_cache, _timing=None):
    hidden_states = np.asarray(hidden_states, np.float32)
    qr = np.asarray(qr, np.float32)
    positions = np.asarray(positions, np.int32)
    W_fused = np.asarray(W_fused, np.float32)
    Wq = np.asarray(Wq, np.float32)
    Wproj = np.asarray(Wproj, np.float32)
    ape = np.asarray(ape, np.float32)
    rms_weight = np.asarray(rms_weight, np.float32)
    cos_sin_cache = np.asarray(cos_sin_cache, np.float32)

    cores = list(range(NC))
    trace = _timing is not None

    # host-side data prep (layout + fp16 hi/lo split; lo scaled by 2^11)
    def _split16(x):
        xh = x.astype(np.float16)
        xl = ((x - xh.astype(np.float32)) * 2048.0).astype(np.float16)
        return np.ascontiguousarray(xh), np.ascontiguousarray(xl)

    hidT_h, hidT_l = _split16(np.ascontiguousarray(hidden_states.T))
    wcomb_h, wcomb_l = _split16(np.ascontiguousarray(
        np.concatenate([W_fused.T, Wproj.T], axis=1)))           # [7168, 576]
    qrT_h, qrT_l = _split16(np.ascontiguousarray(qr.T))          # [1536, T]
    wq_h, wq_l = _split16(np.ascontiguousarray(
        Wq.reshape(H, 128, 12, 128).transpose(0, 3, 2, 1)))      # [h,kk,kc,m]
    csT = np.ascontiguousarray(cos_sin_cache.T)                  # [128, T]
    cc_host = np.ascontiguousarray(np.concatenate([csT[0:64], csT[0:64]]))
    ss_host = np.ascontiguousarray(
        np.concatenate([-csT[64:128], csT[64:128]]))
    perm_host = np.zeros((128, 128), np.float32)
    perm_host[np.arange(128), (np.arange(128) + 64) % 128] = 1.0

    in_la = []
    for i in cores:
        in_la.append({
            "hidt_h": np.ascontiguousarray(hidT_h[KSH * i:KSH * (i + 1)]),
            "hidt_l": np.ascontiguousarray(hidT_l[KSH * i:KSH * (i + 1)]),
            "wcomb_h": np.ascontiguousarray(wcomb_h[KSH * i:KSH * (i + 1)]),
            "wcomb_l": np.ascontiguousarray(wcomb_l[KSH * i:KSH * (i + 1)]),
            "qrt_h": qrT_h, "qrt_l": qrT_l,
            "wq_h": np.ascontiguousarray(wq_h[HSH * i:HSH * (i + 1)]),
            "wq_l": np.ascontiguousarray(wq_l[HSH * i:HSH * (i + 1)]),
            "cc": cc_host, "ss": ss_host, "perm": perm_host,
        })
    r_la = run_bass_kernel_spmd(_get("la"), in_la, cores, trace=trace,
                                trace_cores=cores if trace else None)

    kv_full = np.zeros((576, T), np.float32)
    for i in cores:
        kv_full += r_la.results[i]["kv_part"]
    qT_heads = np.empty((H, 128, T), np.float32)
    for i in cores:
        qT_heads[HSH * i:HSH * (i + 1)] = r_la.results[i]["qro"]

    in_lb = []
    for i in cores:
        kvst = np.zeros((576, 264), np.float32)
        for s in range(2):
            t0 = 256 * i + 128 * s
            lo = t0 - 4
            if lo < 0:
                kvst[:, 132 * s + 4:132 * s + 132] = kv_full[:, 0:t0 + 128]
            else:
                kvst[:, 132 * s:132 * s + 132] = kv_full[:, lo:t0 + 128]
        in_lb.append({
            "kvst": kvst, "ape": ape, "rmsw": rms_weight,
            "cs_k": np.ascontiguousarray(cos_sin_cache[64 * i:64 * i + 64]),
            "haloflag": np.array([NEG if i == 0 else 0.0, 0.0], np.float32),
        })
    r_lb = run_bass_kernel_spmd(_get("lb"), in_lb, cores, trace=trace,
                                trace_cores=cores if trace else None)

    k_full = np.concatenate([r_lb.results[i]["k_loc"] for i in cores], axis=0)
    kT_host = np.ascontiguousarray(k_full.T)                     # [128, C]
    wabs_full = np.concatenate([r_lb.results[i]["wabs"] for i in cores])
    wsgn_full = np.concatenate([r_lb.results[i]["wsgn"] for i in cores])

    in_lc = []
    sels = []
    for i in cores:
        sel = np.concatenate([np.arange(128 * i, 128 * i + 128),
                              np.arange(128 * (15 - i), 128 * (15 - i) + 128)])
        sels.append(sel)
        in_lc.append({
            "qro": np.ascontiguousarray(qT_heads[:, :, sel]),
            "kt": kT_host,
            "wabs": np.ascontiguousarray(wabs_full[sel]),
            "wsgn": np.ascontiguousarray(wsgn_full[sel]),
            "posm3": (positions[sel] - 3).astype(np.float32),
        })
    r_lc = run_bass_kernel_spmd(_get("lc"), in_lc, cores, trace=trace,
                                trace_cores=cores if trace else None)

    out = np.empty((T, TOPK), np.int32)
    for i in cores:
        out[sels[i]] = r_lc.results[i]["out_idx"]

    if _timing is not None:
        _timing["la"] = r_la
        _timing["lb"] = r_lb
        _timing["lc"] = r_lc
    return out


# revision 27
# speedup vs baseline: 1.1969x; 1.1969x over previous
"""DeepseekV4 indexer kernel for 8 trn2 NeuronCores (Bass/Tile).

Three launches, all GEMMs in exact fp32 with N=512 moving dims
(fp32 LOW_HIGH 2-pass runs ~2.2 cyc/row at N=512 vs 4 cyc/row at N=256):

  LA (per core): K-sharded fused kv/score/wts GEMM partial (own 896 HID
     rows x all 2048 tokens, transposed output) + head-sharded q GEMM
     (own 8 heads x all tokens, output [D, T] per head) + RoPE applied
     via a half-swap permutation matmul.  Host reduces the kv partials
     and reshuffles q to token shards.
  LB (per core): the compressor (softmax window -> RMSNorm -> RoPE) on
     its 256-token shard of the reduced kv_score, plus |wts|/sign prep.
     Host gathers compressed K.
  LC (per core): qk matmuls against full K, relu*wts head accumulation
     (Scalar engine: relu(qk*|w|); GpSimd: sign-multiply-accumulate;
     Vector engine reserved for the iterated top-8 extraction), causal
     mask and top-256 in descending-score order.

kernel(**inputs) takes FULL unsharded inputs, returns [2048,256] int32.
"""
import sys
sys.path.insert(0, '/opt/trn_rl_repo')

from contextlib import ExitStack

import numpy as np

import concourse.bass as bass
import concourse.bacc as bacc
import concourse.tile as tile
from concourse import mybir
from concourse.bass_utils import run_bass_kernel_spmd
from concourse.masks import make_identity

T, HID, QR_DIM, H, D, TOPK, R = 2048, 7168, 1536, 64, 128, 256, 4
C = T // R
NC = 8
KSH = HID // NC          # 896 contraction rows per core in LA
HSH = H // NC            # 8 heads per core in LA
EPS = 1e-6
F32 = mybir.dt.float32
F16 = mybir.dt.float16
I32 = mybir.dt.int32
U32 = mybir.dt.uint32
WTS_SCALE = float(H ** -0.5) * float(D ** -0.5)
NEG = -1e30
WIDTHS = (256, 512)      # candidate widths for (low tile j<=7, high tile)

_cache = {}


# --------------------------------------------------------------------------
# LA: kv partial GEMM (K-sharded) + q GEMM (head-sharded) + rope
# --------------------------------------------------------------------------
def _build_la():
    nc = bacc.Bacc()
    hidt_h = nc.declare_dram_parameter("hidt_h", [KSH, T], F16, isOutput=False)
    hidt_l = nc.declare_dram_parameter("hidt_l", [KSH, T], F16, isOutput=False)
    wcomb_h = nc.declare_dram_parameter("wcomb_h", [KSH, 576], F16,
                                        isOutput=False)
    wcomb_l = nc.declare_dram_parameter("wcomb_l", [KSH, 576], F16,
                                        isOutput=False)
    qrt_h = nc.declare_dram_parameter("qrt_h", [QR_DIM, T], F16,
                                      isOutput=False)
    qrt_l = nc.declare_dram_parameter("qrt_l", [QR_DIM, T], F16,
                                      isOutput=False)
    wq_h = nc.declare_dram_parameter("wq_h", [HSH, 128, 12, 128], F16,
                                     isOutput=False)
    wq_l = nc.declare_dram_parameter("wq_l", [HSH, 128, 12, 128], F16,
                                     isOutput=False)
    cc = nc.declare_dram_parameter("cc", [128, T], F32, isOutput=False)
    ss = nc.declare_dram_parameter("ss", [128, T], F32, isOutput=False)
    perm = nc.declare_dram_parameter("perm", [128, 128], F32, isOutput=False)
    kv_part = nc.declare_dram_parameter("kv_part", [576, T], F32,
                                        isOutput=True)
    qro = nc.declare_dram_parameter("qro", [HSH, 128, T], F32, isOutput=True)

    with tile.TileContext(nc) as tc, ExitStack() as ctx:
        const = ctx.enter_context(tc.tile_pool(name="const", bufs=1))
        stage = ctx.enter_context(tc.tile_pool(name="stage", bufs=2))
        qrp = ctx.enter_context(tc.tile_pool(name="qrp", bufs=2))

        perm_sb = const.tile([128, 128], F32)
        nc.sync.dma_start(out=perm_sb, in_=perm[:])
        cc_sb = const.tile([128, T], F32)
        nc.sync.dma_start(out=cc_sb, in_=cc[:])
        ss_sb = const.tile([128, T], F32)
        nc.sync.dma_start(out=ss_sb, in_=ss[:])

        # all 8 heads' Wq hi/lo tiles resident: [128, 12, 128] f16 (3KB/part)
        wqt = []
        for m in range(HSH):
            th = const.tile([128, 12, 128], F16, tag=f"wqh{m}", name=f"wqh{m}")
            nc.sync.dma_start(out=th, in_=wq_h[m])
            tl = const.tile([128, 12, 128], F16, tag=f"wql{m}", name=f"wql{m}")
            nc.sync.dma_start(out=tl, in_=wq_l[m])
            wqt.append((th, tl))

        # ---- kv/score/wts partial GEMM: out [576, 2048] (transposed) ----
        with tc.tile_pool(name="wcp", bufs=1) as wcp, \
             tc.tile_pool(name="hidp", bufs=14) as hidp, \
             tc.tile_pool(name="kvps", bufs=2, space="PSUM") as kvps, \
             tc.tile_pool(name="kvo", bufs=3) as kvo:
            wct = []
            for kc in range(7):
                th = wcp.tile([128, 576], F16, tag=f"wch{kc}", name=f"wch{kc}")
                nc.sync.dma_start(out=th,
                                  in_=wcomb_h[kc * 128:(kc + 1) * 128, :])
                tl = wcp.tile([128, 576], F16, tag=f"wcl{kc}", name=f"wcl{kc}")
                nc.sync.dma_start(out=tl,
                                  in_=wcomb_l[kc * 128:(kc + 1) * 128, :])
                wct.append((th, tl))
            for b in range(4):
                hbt = []
                for kc in range(7):
                    ht = hidp.tile([128, 512], F16, tag="hidh")
                    nc.sync.dma_start(
                        out=ht,
                        in_=hidt_h[kc * 128:(kc + 1) * 128,
                                   b * 512:(b + 1) * 512])
                    hl = hidp.tile([128, 512], F16, tag="hidl")
                    nc.sync.dma_start(
                        out=hl,
                        in_=hidt_l[kc * 128:(kc + 1) * 128,
                                   b * 512:(b + 1) * 512])
                    hbt.append((ht, hl))
                for m in range(5):
                    M = 128 if m < 4 else 64
                    ms = slice(m * 128, m * 128 + M)
                    p1 = kvps.tile([128, 512], F32, tag="kv1")
                    p2 = kvps.tile([128, 512], F32, tag="kv2")
                    for kc in range(7):
                        nc.tensor.matmul(p1[:M, :], wct[kc][0][:, ms],
                                         hbt[kc][0], start=(kc == 0),
                                         stop=(kc == 6))
                    for kc in range(7):
                        nc.tensor.matmul(p2[:M, :], wct[kc][0][:, ms],
                                         hbt[kc][1], start=(kc == 0),
                                         stop=False)
                        nc.tensor.matmul(p2[:M, :], wct[kc][1][:, ms],
                                         hbt[kc][0], start=False,
                                         stop=(kc == 6))
                    ob = kvo.tile([128, 512], F32, tag="kvo")
                    nc.scalar.copy(ob[:M, :], p1[:M, :])
                    nc.vector.scalar_tensor_tensor(
                        out=ob[:M, :], in0=p2[:M, :], scalar=1.0 / 2048.0,
                        in1=ob[:M, :], op0=mybir.AluOpType.mult,
                        op1=mybir.AluOpType.add)
                    nc.sync.dma_start(
                        out=kv_part[m * 128:m * 128 + M,
                                    b * 512:(b + 1) * 512],
                        in_=ob[:M, :])

        # ---- q GEMM (8 heads) + rope, quarter-streamed over tokens ----
        qps = ctx.enter_context(tc.tile_pool(name="qps", bufs=2,
                                             space="PSUM"))
        swps = ctx.enter_context(tc.tile_pool(name="swps", bufs=2,
                                              space="PSUM"))
        for qtr in range(4):
            qth = qrp.tile([128, 12, 512], F16, tag="qrh")
            nc.sync.dma_start(
                out=qth,
                in_=qrt_h[:].rearrange("(kc p) t -> p kc t", p=128)[
                    :, :, qtr * 512:(qtr + 1) * 512])
            qtl = qrp.tile([128, 12, 512], F16, tag="qrl")
            nc.sync.dma_start(
                out=qtl,
                in_=qrt_l[:].rearrange("(kc p) t -> p kc t", p=128)[
                    :, :, qtr * 512:(qtr + 1) * 512])
            for m in range(HSH):
                p1 = qps.tile([128, 512], F32, tag="q1")
                p2 = qps.tile([128, 512], F32, tag="q2")
                for kc in range(12):
                    nc.tensor.matmul(p1, wqt[m][0][:, kc, :], qth[:, kc, :],
                                     start=(kc == 0), stop=(kc == 11))
                for kc in range(12):
                    nc.tensor.matmul(p2, wqt[m][0][:, kc, :], qtl[:, kc, :],
                                     start=(kc == 0), stop=False)
                    nc.tensor.matmul(p2, wqt[m][1][:, kc, :], qth[:, kc, :],
                                     start=False, stop=(kc == 11))
                q_sb = stage.tile([128, 512], F32, tag="qsb")
                nc.scalar.copy(q_sb, p1)
                nc.vector.scalar_tensor_tensor(
                    out=q_sb, in0=p2, scalar=1.0 / 2048.0, in1=q_sb,
                    op0=mybir.AluOpType.mult, op1=mybir.AluOpType.add)
                ps_sw = swps.tile([128, 512], F32, tag="sw")
                nc.tensor.matmul(ps_sw, perm_sb, q_sb, start=True, stop=True)
                o = qtr * 512
                t1 = stage.tile([128, 512], F32, tag="t1")
                nc.vector.tensor_mul(t1, q_sb, cc_sb[:, o:o + 512])
                t2 = stage.tile([128, 512], F32, tag="t2")
                nc.vector.tensor_mul(t2, ps_sw, ss_sb[:, o:o + 512])
                qo = stage.tile([128, 512], F32, tag="qo")
                nc.vector.tensor_add(qo, t1, t2)
                nc.sync.dma_start(out=qro[m][:, o:o + 512], in_=qo)

    nc.finalize()
    return nc


# --------------------------------------------------------------------------
# LB: compressor on own 256-token shard of reduced kv_score + wts prep
# --------------------------------------------------------------------------
def _build_lb():
    nc = bacc.Bacc()
    kvst = nc.declare_dram_parameter("kvst", [576, 264], F32, isOutput=False)
    ape = nc.declare_dram_parameter("ape", [8, D], F32, isOutput=False)
    rmsw = nc.declare_dram_parameter("rmsw", [D], F32, isOutput=False)
    cs_k = nc.declare_dram_parameter("cs_k", [64, D], F32, isOutput=False)
    haloflag = nc.declare_dram_parameter("haloflag", [2], F32, isOutput=False)
    k_loc = nc.declare_dram_parameter("k_loc", [64, D], F32, isOutput=True)
    wabs = nc.declare_dram_parameter("wabs", [256, H], F32, isOutput=True)
    wsgn = nc.declare_dram_parameter("wsgn", [256, H], F32, isOutput=True)

    with tile.TileContext(nc) as tc, ExitStack() as ctx:
        const = ctx.enter_context(tc.tile_pool(name="const", bufs=1))
        work = ctx.enter_context(tc.tile_pool(name="work", bufs=2))

        ident = const.tile([128, 128], F32)
        make_identity(nc, ident)

        def tp(ps_out, in_sb):
            p = in_sb.shape[0]
            nc.tensor.transpose(ps_out, in_sb, ident[:p, :p])

        # kv/score tiles straight from DRAM (rows of kvst)
        kvt = []
        for r4 in range(4):
            t = const.tile([128, 264], F32, tag=f"kv{r4}", name=f"kv{r4}")
            nc.sync.dma_start(out=t, in_=kvst[r4 * 128:(r4 + 1) * 128, :])
            kvt.append(t)
        kv_old, kv_new, sc_old, sc_new = kvt
        wts_sb = const.tile([64, 264], F32)
        nc.sync.dma_start(out=wts_sb, in_=kvst[512:576, :])

        with tc.tile_pool(name="tpsB", bufs=2, space="PSUM") as tpsB:
            # wts -> [t, h]; emit |w|*scale and sign
            for s in range(2):
                ps = tpsB.tile([128, 64], F32, tag="wtp")
                tp(ps, wts_sb[:, 4 + 132 * s:132 + 132 * s])
                wa = work.tile([128, 64], F32, tag="wa")
                nc.scalar.activation(wa, ps, mybir.ActivationFunctionType.Abs,
                                     scale=WTS_SCALE)
                nc.sync.dma_start(out=wabs[128 * s:128 * (s + 1), :], in_=wa)
                sg = work.tile([128, 64], F32, tag="sg")
                nc.scalar.activation(sg, ps,
                                     mybir.ActivationFunctionType.Sign)
                nc.sync.dma_start(out=wsgn[128 * s:128 * (s + 1), :], in_=sg)

            # ape transposed + replicated [128, 32, 8]
            ape_st = work.tile([8, D], F32, tag="ape_st")
            nc.sync.dma_start(out=ape_st, in_=ape[:])
            aps = tpsB.tile([128, 8], F32, tag="apetp")
            tp(aps, ape_st)
            apeT = const.tile([128, 8], F32)
            nc.scalar.copy(apeT, aps)
            ape_rep = const.tile([128, 32, 8], F32)
            for g in range(32):
                nc.vector.tensor_copy(ape_rep[:, g, :], apeT)

            rms_rep = const.tile([32, D], F32)
            nc.sync.dma_start(out=rms_rep, in_=bass.AP(
                tensor=rmsw, offset=0, ap=[[0, 32], [1, D]]))

            cs_st = []
            for s in range(2):
                cst = const.tile([32, D], F32, tag=f"cs{s}", name=f"cs{s}")
                nc.sync.dma_start(out=cst, in_=cs_k[32 * s:32 * s + 32, :])
                cs_st.append(cst)

            hf = []
            for s in range(2):
                h = const.tile([128, 1], F32, tag=f"hf{s}")
                nc.sync.dma_start(out=h, in_=bass.AP(
                    tensor=haloflag, offset=s, ap=[[0, 128], [1, 1]]))
                hf.append(h)

            for s in range(2):
                o = 132 * s
                gates = work.tile([128, 32, 8], F32, tag="gates")
                so_v = sc_old[:, o:o + 128].rearrange("p (g x) -> p g x", x=4)
                sn_v = sc_new[:, o + 4:o + 132].rearrange(
                    "p (g x) -> p g x", x=4)
                ko_v = kv_old[:, o:o + 128].rearrange("p (g x) -> p g x", x=4)
                kn_v = kv_new[:, o + 4:o + 132].rearrange(
                    "p (g x) -> p g x", x=4)
                nc.vector.tensor_add(gates[:, :, 0:4], so_v,
                                     ape_rep[:, :, 0:4])
                nc.vector.tensor_add(gates[:, :, 4:8], sn_v,
                                     ape_rep[:, :, 4:8])
                nc.vector.tensor_scalar(gates[:, 0, 0:4], gates[:, 0, 0:4],
                                        hf[s], None, op0=mybir.AluOpType.add)
                gmax = work.tile([128, 32], F32, tag="gmax")
                nc.vector.reduce_max(gmax, gates, axis=mybir.AxisListType.X)
                nc.vector.tensor_sub(gates, gates,
                                     gmax.to_broadcast([128, 32, 8]))
                ex = work.tile([128, 32, 8], F32, tag="ex")
                nc.scalar.activation(ex, gates,
                                     mybir.ActivationFunctionType.Exp)
                den = work.tile([128, 32], F32, tag="den")
                nc.vector.reduce_sum(den, ex, axis=mybir.AxisListType.X)
                rec = work.tile([128, 32], F32, tag="rec")
                nc.vector.reciprocal(rec, den)
                w8 = work.tile([128, 32, 8], F32, tag="w8")
                nc.vector.tensor_mul(w8, ex, rec.to_broadcast([128, 32, 8]))
                prod = work.tile([128, 32, 8], F32, tag="prod")
                nc.vector.tensor_mul(prod[:, :, 0:4], w8[:, :, 0:4], ko_v)
                nc.vector.tensor_mul(prod[:, :, 4:8], w8[:, :, 4:8], kn_v)
                comp = work.tile([128, 32], F32, tag="comp")
                nc.vector.reduce_sum(comp, prod, axis=mybir.AxisListType.X)

                cps = tpsB.tile([32, 128], F32, tag="ctp")
                tp(cps, comp)
                compT = work.tile([32, D], F32, tag="compT")
                nc.scalar.copy(compT, cps)

                sq = work.tile([32, D], F32, tag="sq")
                nc.vector.tensor_mul(sq, compT, compT)
                ssum = work.tile([32, 1], F32, tag="ssum")
                nc.vector.reduce_sum(ssum, sq, axis=mybir.AxisListType.X)
                nc.vector.tensor_scalar(ssum, ssum, 1.0 / D, EPS,
                                        op0=mybir.AluOpType.mult,
                                        op1=mybir.AluOpType.add)
                rt = work.tile([32, 1], F32, tag="rt")
                nc.scalar.sqrt(rt, ssum)
                rs = work.tile([32, 1], F32, tag="rs")
                nc.vector.reciprocal(rs, rt)
                nc.vector.tensor_scalar(compT, compT, rs, None,
                                        op0=mybir.AluOpType.mult)
                nc.vector.tensor_mul(compT, compT, rms_rep)

                co = cs_st[s][:, 0:64]
                si = cs_st[s][:, 64:128]
                x1 = compT[:, 0:64]
                x2 = compT[:, 64:128]
                tmp = work.tile([32, D], F32, tag="ktmp")
                kx = work.tile([32, D], F32, tag="kx")
                nc.vector.tensor_mul(kx[:, 0:64], x1, co)
                nc.vector.tensor_mul(tmp[:, 0:64], x2, si)
                nc.vector.tensor_sub(kx[:, 0:64], kx[:, 0:64], tmp[:, 0:64])
                nc.vector.tensor_mul(kx[:, 64:128], x2, co)
                nc.vector.tensor_mul(tmp[:, 64:128], x1, si)
                nc.vector.tensor_add(kx[:, 64:128], kx[:, 64:128],
                                     tmp[:, 64:128])
                nc.sync.dma_start(out=k_loc[32 * s:32 * s + 32, :], in_=kx)

    nc.finalize()
    return nc


# --------------------------------------------------------------------------
# LC: qk + relu*wts head-sum (ACT+GpSimd) + causal mask + top-k (DVE)
# --------------------------------------------------------------------------
def _build_lc():
    nc = bacc.Bacc()
    qro = nc.declare_dram_parameter("qro", [H, 128, 256], F32, isOutput=False)
    kt = nc.declare_dram_parameter("kt", [128, C], F32, isOutput=False)
    wabs = nc.declare_dram_parameter("wabs", [256, H], F32, isOutput=False)
    wsgn = nc.declare_dram_parameter("wsgn", [256, H], F32, isOutput=False)
    posm3 = nc.declare_dram_parameter("posm3", [256], F32, isOutput=False)
    out_idx = nc.declare_dram_parameter("out_idx", [256, TOPK], I32,
                                        isOutput=True)

    with tile.TileContext(nc) as tc, ExitStack() as ctx:
        const = ctx.enter_context(tc.tile_pool(name="const", bufs=1))
        work = ctx.enter_context(tc.tile_pool(name="work", bufs=2))
        tk = ctx.enter_context(tc.tile_pool(name="tk", bufs=2))
        yp = ctx.enter_context(tc.tile_pool(name="yp", bufs=3))
        qkps = ctx.enter_context(tc.tile_pool(name="qkps", bufs=3,
                                              space="PSUM"))

        kt_sb = const.tile([128, C], F32)
        nc.sync.dma_start(out=kt_sb, in_=kt[:])
        qrop = ctx.enter_context(tc.tile_pool(name="qrop", bufs=1))

        wabs_sb, wsgn_sb, pos_sb = [], [], []
        for tt in range(2):
            wa = const.tile([128, H], F32, tag=f"wa{tt}", name=f"wa{tt}")
            nc.sync.dma_start(out=wa, in_=wabs[tt * 128:(tt + 1) * 128, :])
            wabs_sb.append(wa)
            sg = const.tile([128, H], F32, tag=f"sg{tt}", name=f"sg{tt}")
            nc.sync.dma_start(out=sg, in_=wsgn[tt * 128:(tt + 1) * 128, :])
            wsgn_sb.append(sg)
            p = const.tile([128, 1], F32, tag=f"pos{tt}", name=f"pos{tt}")
            nc.sync.dma_start(out=p, in_=posm3[tt * 128:(tt + 1) * 128])
            pos_sb.append(p)

        c4p = const.tile([128, C], F32)
        nc.gpsimd.iota(c4p, pattern=[[4, C]], base=0, channel_multiplier=0,
                       allow_small_or_imprecise_dtypes=True)
        c4f = const.tile([128, C], F32)
        nc.vector.tensor_scalar(c4f, c4p, -1.0, None,
                                op0=mybir.AluOpType.mult)
        negs = const.tile([128, C], F32)
        nc.vector.memset(negs, NEG)
        neg1 = const.tile([128, TOPK], I32)
        nc.vector.memset(neg1, -1)

        acc = [const.tile([128, WIDTHS[tt]], F32, tag=f"acc{tt}",
                          name=f"acc{tt}") for tt in range(2)]

        def emit_topk(tt):
            W = WIDTHS[tt]
            cmp = work.tile([128, C], F32, tag="cmp", name="cmp")
            nc.vector.tensor_scalar(cmp[:, :W], c4f[:, :W], pos_sb[tt], None,
                                    op0=mybir.AluOpType.add)
            mbit = work.tile([128, C], U32, tag="mbit", name="mbit")
            nc.vector.tensor_scalar(mbit[:, :W], cmp[:, :W], 0.0, None,
                                    op0=mybir.AluOpType.is_lt)
            nc.vector.copy_predicated(acc[tt], mbit[:, :W], negs[:, :W])

            idx = tk.tile([128, TOPK], U32, tag="idx", name="idx")
            vals = acc[tt]
            for it in range(32):
                mx = tk.tile([128, 8], F32, tag="mx", name="mx")
                nc.vector.max(out=mx, in_=vals)
                nc.vector.max_index(out=idx[:, it * 8:(it + 1) * 8],
                                    in_max=mx, in_values=vals)
                nc.vector.match_replace(out=vals, in_to_replace=mx,
                                        in_values=vals, imm_value=NEG)

            idx32 = tk.tile([128, TOPK], I32, tag="idx32", name="idx32")
            nc.vector.tensor_copy(idx32, idx)
            rmp = work.tile([128, TOPK], F32, tag="rmp", name="rmp")
            nc.vector.tensor_scalar(rmp, c4f[:, :TOPK], pos_sb[tt], None,
                                    op0=mybir.AluOpType.add)
            rbit = work.tile([128, TOPK], U32, tag="rbit", name="rbit")
            nc.vector.tensor_scalar(rbit, rmp, 0.0, None,
                                    op0=mybir.AluOpType.is_lt)
            nc.vector.copy_predicated(idx32, rbit, neg1)
            nc.sync.dma_start(out=out_idx[tt * 128:(tt + 1) * 128, :],
                              in_=idx32)

        # per-head q tiles resident (loaded once, used by both tt passes)
        qts = []
        for h in range(H):
            qt = qrop.tile([128, 256], F32, tag=f"qro{h}", name=f"qro{h}")
            nc.sync.dma_start(out=qt, in_=qro[h])
            qts.append(qt)
        for tt in range(2):
            W = WIDTHS[tt]
            nc.vector.memset(acc[tt], 0.0)
            for h in range(H):
                ps_qk = qkps.tile([128, 512], F32, tag="qk", name="ps_qk")
                nc.tensor.matmul(ps_qk[:, :W],
                                 qts[h][:, tt * 128:(tt + 1) * 128],
                                 kt_sb[:, :W], start=True, stop=True)
                y = yp.tile([128, 512], F32, tag="y")
                nc.scalar.activation(y[:, :W], ps_qk[:, :W],
                                     mybir.ActivationFunctionType.Relu,
                                     scale=wabs_sb[tt][:, h:h + 1])
                nc.vector.scalar_tensor_tensor(
                    out=acc[tt], in0=y[:, :W], scalar=wsgn_sb[tt][:, h:h + 1],
                    in1=acc[tt], op0=mybir.AluOpType.mult,
                    op1=mybir.AluOpType.add)
            emit_topk(tt)

    nc.finalize()
    return nc


def _get(name):
    if name not in _cache:
        _cache[name] = {"la": _build_la, "lb": _build_lb,
                        "lc": _build_lc}[name]()
    return _cache[name]


def kernel(hidden_states, qr, positions, W_fused, Wq, Wproj, ape, rms_weight,
           cos_sin_cache, _timing=None):
    hidden_states = np.asarray(hidden_states, np.float32)
    qr = np.asarray(qr, np.float32)
    positions = np.asarray(positions, np.int32)
    W_fused = np.asarray(W_fused, np.float32)
    Wq = np.asarray(Wq, np.float32)
    Wproj = np.asarray(Wproj, np.float32)
    ape = np.asarray(ape, np.float32)
    rms_weight = np.asarray(rms_weight, np.float32)
    cos_sin_cache = np.asarray(cos_sin_cache, np.float32)

    cores = list(range(NC))
    trace = _timing is not None

    # host-side data prep (layout + fp16 hi/lo split; lo scaled by 2^11)
    def _split16(x):
        xh = x.astype(np.float16)
        xl = ((x - xh.astype(np.float32)) * 2048.0).astype(np.float16)
        return np.ascontiguousarray(xh), np.ascontiguousarray(xl)

    hidT_h, hidT_l = _split16(np.ascontiguousarray(hidden_states.T))
    wcomb_h, wcomb_l = _split16(np.ascontiguousarray(
        np.concatenate([W_fused.T, Wproj.T], axis=1)))           # [7168, 576]
    qrT_h, qrT_l = _split16(np.ascontiguousarray(qr.T))          # [1536, T]
    wq_h, wq_l = _split16(np.ascontiguousarray(
        Wq.reshape(H, 128, 12, 128).transpose(0, 3, 2, 1)))      # [h,kk,kc,m]
    csT = np.ascontiguousarray(cos_sin_cache.T)                  # [128, T]
    cc_host = np.ascontiguousarray(np.concatenate([csT[0:64], csT[0:64]]))
    ss_host = np.ascontiguousarray(
        np.concatenate([-csT[64:128], csT[64:128]]))
    perm_host = np.zeros((128, 128), np.float32)
    perm_host[np.arange(128), (np.arange(128) + 64) % 128] = 1.0

    in_la = []
    for i in cores:
        in_la.append({
            "hidt_h": np.ascontiguousarray(hidT_h[KSH * i:KSH * (i + 1)]),
            "hidt_l": np.ascontiguousarray(hidT_l[KSH * i:KSH * (i + 1)]),
            "wcomb_h": np.ascontiguousarray(wcomb_h[KSH * i:KSH * (i + 1)]),
            "wcomb_l": np.ascontiguousarray(wcomb_l[KSH * i:KSH * (i + 1)]),
            "qrt_h": qrT_h, "qrt_l": qrT_l,
            "wq_h": np.ascontiguousarray(wq_h[HSH * i:HSH * (i + 1)]),
            "wq_l": np.ascontiguousarray(wq_l[HSH * i:HSH * (i + 1)]),
            "cc": cc_host, "ss": ss_host, "perm": perm_host,
        })
    r_la = run_bass_kernel_spmd(_get("la"), in_la, cores, trace=trace,
                                trace_cores=cores if trace else None)

    kv_full = np.zeros((576, T), np.float32)
    for i in cores:
        kv_full += r_la.results[i]["kv_part"]
    qT_heads = np.empty((H, 128, T), np.float32)
    for i in cores:
        qT_heads[HSH * i:HSH * (i + 1)] = r_la.results[i]["qro"]

    in_lb = []
    for i in cores:
        kvst = np.zeros((576, 264), np.float32)
        for s in range(2):
            t0 = 256 * i + 128 * s
            lo = t0 - 4
            if lo < 0:
                kvst[:, 132 * s + 4:132 * s + 132] = kv_full[:, 0:t0 + 128]
            else:
                kvst[:, 132 * s:132 * s + 132] = kv_full[:, lo:t0 + 128]
        in_lb.append({
            "kvst": kvst, "ape": ape, "rmsw": rms_weight,
            "cs_k": np.ascontiguousarray(cos_sin_cache[64 * i:64 * i + 64]),
            "haloflag": np.array([NEG if i == 0 else 0.0, 0.0], np.float32),
        })
    r_lb = run_bass_kernel_spmd(_get("lb"), in_lb, cores, trace=trace,
                                trace_cores=cores if trace else None)

    k_full = np.concatenate([r_lb.results[i]["k_loc"] for i in cores], axis=0)
    kT_host = np.ascontiguousarray(k_full.T)                     # [128, C]
    wabs_full = np.concatenate([r_lb.results[i]["wabs"] for i in cores])
    wsgn_full = np.concatenate([r_lb.results[i]["wsgn"] for i in cores])

    in_lc = []
    sels = []
    for i in cores:
        sel = np.concatenate([np.arange(128 * i, 128 * i + 128),
                              np.arange(128 * (15 - i), 128 * (15 - i) + 128)])
        sels.append(sel)
        in_lc.append({
            "qro": np.ascontiguousarray(qT_heads[:, :, sel]),
            "kt": kT_host,
            "wabs": np.ascontiguousarray(wabs_full[sel]),
            "wsgn": np.ascontiguousarray(wsgn_full[sel]),
            "posm3": (positions[sel] - 3).astype(np.float32),
        })
    r_lc = run_bass_kernel_spmd(_get("lc"), in_lc, cores, trace=trace,
                                trace_cores=cores if trace else None)

    out = np.empty((T, TOPK), np.int32)
    for i in cores:
        out[sels[i]] = r_lc.results[i]["out_idx"]

    if _timing is not None:
        _timing["la"] = r_la
        _timing["lb"] = r_lb
        _timing["lc"] = r_lc
    return out


# revision 28
# speedup vs baseline: 1.2421x; 1.0378x over previous
"""DeepseekV4 indexer kernel for 8 trn2 NeuronCores (Bass/Tile).

Three launches, all GEMMs in exact fp32 with N=512 moving dims
(fp32 LOW_HIGH 2-pass runs ~2.2 cyc/row at N=512 vs 4 cyc/row at N=256):

  LA (per core): K-sharded fused kv/score/wts GEMM partial (own 896 HID
     rows x all 2048 tokens, transposed output) + head-sharded q GEMM
     (own 8 heads x all tokens, output [D, T] per head) + RoPE applied
     via a half-swap permutation matmul.  Host reduces the kv partials
     and reshuffles q to token shards.
  LB (per core): the compressor (softmax window -> RMSNorm -> RoPE) on
     its 256-token shard of the reduced kv_score, plus |wts|/sign prep.
     Host gathers compressed K.
  LC (per core): qk matmuls against full K, relu*wts head accumulation
     (Scalar engine: relu(qk*|w|); GpSimd: sign-multiply-accumulate;
     Vector engine reserved for the iterated top-8 extraction), causal
     mask and top-256 in descending-score order.

kernel(**inputs) takes FULL unsharded inputs, returns [2048,256] int32.
"""
import sys
sys.path.insert(0, '/opt/trn_rl_repo')

from contextlib import ExitStack

import numpy as np

import concourse.bass as bass
import concourse.bacc as bacc
import concourse.tile as tile
from concourse import mybir
from concourse.bass_utils import run_bass_kernel_spmd
from concourse.masks import make_identity

T, HID, QR_DIM, H, D, TOPK, R = 2048, 7168, 1536, 64, 128, 256, 4
C = T // R
NC = 8
KSH = HID // NC          # 896 contraction rows per core in LA
HSH = H // NC            # 8 heads per core in LA
EPS = 1e-6
F32 = mybir.dt.float32
F16 = mybir.dt.float16
I32 = mybir.dt.int32
U32 = mybir.dt.uint32
WTS_SCALE = float(H ** -0.5) * float(D ** -0.5)
NEG = -1e30
WIDTHS = (256, 512)      # candidate widths for (low tile j<=7, high tile)

_cache = {}


# --------------------------------------------------------------------------
# LA: kv partial GEMM (K-sharded) + q GEMM (head-sharded) + rope
# --------------------------------------------------------------------------
def _build_la():
    nc = bacc.Bacc()
    hidt_h = nc.declare_dram_parameter("hidt_h", [KSH, T], F16, isOutput=False)
    hidt_l = nc.declare_dram_parameter("hidt_l", [KSH, T], F16, isOutput=False)
    wcomb_h = nc.declare_dram_parameter("wcomb_h", [KSH, 576], F16,
                                        isOutput=False)
    wcomb_l = nc.declare_dram_parameter("wcomb_l", [KSH, 576], F16,
                                        isOutput=False)
    qrt_h = nc.declare_dram_parameter("qrt_h", [QR_DIM, T], F16,
                                      isOutput=False)
    qrt_l = nc.declare_dram_parameter("qrt_l", [QR_DIM, T], F16,
                                      isOutput=False)
    wq_h = nc.declare_dram_parameter("wq_h", [HSH, 128, 12, 128], F16,
                                     isOutput=False)
    wq_l = nc.declare_dram_parameter("wq_l", [HSH, 128, 12, 128], F16,
                                     isOutput=False)
    cc = nc.declare_dram_parameter("cc", [128, T], F32, isOutput=False)
    ss = nc.declare_dram_parameter("ss", [128, T], F32, isOutput=False)
    perm = nc.declare_dram_parameter("perm", [128, 128], F32, isOutput=False)
    kv_part = nc.declare_dram_parameter("kv_part", [576, T], F32,
                                        isOutput=True)
    qro = nc.declare_dram_parameter("qro", [HSH, 128, T], F32, isOutput=True)

    with tile.TileContext(nc) as tc, ExitStack() as ctx:
        const = ctx.enter_context(tc.tile_pool(name="const", bufs=1))
        stage = ctx.enter_context(tc.tile_pool(name="stage", bufs=2))
        qrp = ctx.enter_context(tc.tile_pool(name="qrp", bufs=2))

        perm_sb = const.tile([128, 128], F32)
        nc.sync.dma_start(out=perm_sb, in_=perm[:])
        cc_sb = const.tile([128, T], F32)
        nc.sync.dma_start(out=cc_sb, in_=cc[:])
        ss_sb = const.tile([128, T], F32)
        nc.sync.dma_start(out=ss_sb, in_=ss[:])

        # all 8 heads' Wq hi/lo tiles resident: [128, 12, 128] f16 (3KB/part)
        wqt = []
        for m in range(HSH):
            th = const.tile([128, 12, 128], F16, tag=f"wqh{m}", name=f"wqh{m}")
            nc.sync.dma_start(out=th, in_=wq_h[m])
            tl = const.tile([128, 12, 128], F16, tag=f"wql{m}", name=f"wql{m}")
            nc.sync.dma_start(out=tl, in_=wq_l[m])
            wqt.append((th, tl))

        # ---- kv/score/wts partial GEMM: out [576, 2048] (transposed) ----
        with tc.tile_pool(name="wcp", bufs=1) as wcp, \
             tc.tile_pool(name="hidp", bufs=14) as hidp, \
             tc.tile_pool(name="kvps", bufs=2, space="PSUM") as kvps, \
             tc.tile_pool(name="kvo", bufs=3) as kvo:
            wct = []
            for kc in range(7):
                th = wcp.tile([128, 576], F16, tag=f"wch{kc}", name=f"wch{kc}")
                nc.sync.dma_start(out=th,
                                  in_=wcomb_h[kc * 128:(kc + 1) * 128, :])
                tl = wcp.tile([128, 576], F16, tag=f"wcl{kc}", name=f"wcl{kc}")
                nc.sync.dma_start(out=tl,
                                  in_=wcomb_l[kc * 128:(kc + 1) * 128, :])
                wct.append((th, tl))
            for b in range(4):
                hbt = []
                for kc in range(7):
                    ht = hidp.tile([128, 512], F16, tag="hidh")
                    nc.sync.dma_start(
                        out=ht,
                        in_=hidt_h[kc * 128:(kc + 1) * 128,
                                   b * 512:(b + 1) * 512])
                    hl = hidp.tile([128, 512], F16, tag="hidl")
                    nc.sync.dma_start(
                        out=hl,
                        in_=hidt_l[kc * 128:(kc + 1) * 128,
                                   b * 512:(b + 1) * 512])
                    hbt.append((ht, hl))
                for m in range(5):
                    M = 128 if m < 4 else 64
                    ms = slice(m * 128, m * 128 + M)
                    p1 = kvps.tile([128, 512], F32, tag="kv1")
                    p2 = kvps.tile([128, 512], F32, tag="kv2")
                    for kc in range(7):
                        nc.tensor.matmul(p1[:M, :], wct[kc][0][:, ms],
                                         hbt[kc][0], start=(kc == 0),
                                         stop=(kc == 6))
                    for kc in range(7):
                        nc.tensor.matmul(p2[:M, :], wct[kc][0][:, ms],
                                         hbt[kc][1], start=(kc == 0),
                                         stop=False)
                        nc.tensor.matmul(p2[:M, :], wct[kc][1][:, ms],
                                         hbt[kc][0], start=False,
                                         stop=(kc == 6))
                    ob = kvo.tile([128, 512], F32, tag="kvo")
                    nc.scalar.copy(ob[:M, :], p1[:M, :])
                    nc.vector.scalar_tensor_tensor(
                        out=ob[:M, :], in0=p2[:M, :], scalar=1.0 / 2048.0,
                        in1=ob[:M, :], op0=mybir.AluOpType.mult,
                        op1=mybir.AluOpType.add)
                    nc.sync.dma_start(
                        out=kv_part[m * 128:m * 128 + M,
                                    b * 512:(b + 1) * 512],
                        in_=ob[:M, :])

        # ---- q GEMM (8 heads) + rope, quarter-streamed over tokens ----
        qps = ctx.enter_context(tc.tile_pool(name="qps", bufs=2,
                                             space="PSUM"))
        swps = ctx.enter_context(tc.tile_pool(name="swps", bufs=2,
                                              space="PSUM"))
        for qtr in range(4):
            qth = qrp.tile([128, 12, 512], F16, tag="qrh")
            nc.sync.dma_start(
                out=qth,
                in_=qrt_h[:].rearrange("(kc p) t -> p kc t", p=128)[
                    :, :, qtr * 512:(qtr + 1) * 512])
            qtl = qrp.tile([128, 12, 512], F16, tag="qrl")
            nc.sync.dma_start(
                out=qtl,
                in_=qrt_l[:].rearrange("(kc p) t -> p kc t", p=128)[
                    :, :, qtr * 512:(qtr + 1) * 512])
            for m in range(HSH):
                p1 = qps.tile([128, 512], F32, tag="q1")
                p2 = qps.tile([128, 512], F32, tag="q2")
                for kc in range(12):
                    nc.tensor.matmul(p1, wqt[m][0][:, kc, :], qth[:, kc, :],
                                     start=(kc == 0), stop=(kc == 11))
                for kc in range(12):
                    nc.tensor.matmul(p2, wqt[m][0][:, kc, :], qtl[:, kc, :],
                                     start=(kc == 0), stop=False)
                    nc.tensor.matmul(p2, wqt[m][1][:, kc, :], qth[:, kc, :],
                                     start=False, stop=(kc == 11))
                q_sb = stage.tile([128, 512], F32, tag="qsb")
                nc.scalar.copy(q_sb, p1)
                nc.vector.scalar_tensor_tensor(
                    out=q_sb, in0=p2, scalar=1.0 / 2048.0, in1=q_sb,
                    op0=mybir.AluOpType.mult, op1=mybir.AluOpType.add)
                q_sw = stage.tile([128, 512], F32, tag="qsw")
                nc.sync.dma_start(out=q_sw[0:64, :], in_=q_sb[64:128, :])
                nc.sync.dma_start(out=q_sw[64:128, :], in_=q_sb[0:64, :])
                o = qtr * 512
                t1 = stage.tile([128, 512], F32, tag="t1")
                nc.vector.tensor_mul(t1, q_sb, cc_sb[:, o:o + 512])
                t2 = stage.tile([128, 512], F32, tag="t2")
                nc.vector.tensor_mul(t2, q_sw, ss_sb[:, o:o + 512])
                qo = stage.tile([128, 512], F32, tag="qo")
                nc.gpsimd.tensor_add(qo, t1, t2)
                nc.sync.dma_start(out=qro[m][:, o:o + 512], in_=qo)

    nc.finalize()
    return nc


# --------------------------------------------------------------------------
# LB: compressor on own 256-token shard of reduced kv_score + wts prep
# --------------------------------------------------------------------------
def _build_lb():
    nc = bacc.Bacc()
    kvst = nc.declare_dram_parameter("kvst", [576, 264], F32, isOutput=False)
    ape = nc.declare_dram_parameter("ape", [8, D], F32, isOutput=False)
    rmsw = nc.declare_dram_parameter("rmsw", [D], F32, isOutput=False)
    cs_k = nc.declare_dram_parameter("cs_k", [64, D], F32, isOutput=False)
    haloflag = nc.declare_dram_parameter("haloflag", [2], F32, isOutput=False)
    k_loc = nc.declare_dram_parameter("k_loc", [64, D], F32, isOutput=True)
    wabs = nc.declare_dram_parameter("wabs", [256, H], F32, isOutput=True)
    wsgn = nc.declare_dram_parameter("wsgn", [256, H], F32, isOutput=True)

    with tile.TileContext(nc) as tc, ExitStack() as ctx:
        const = ctx.enter_context(tc.tile_pool(name="const", bufs=1))
        work = ctx.enter_context(tc.tile_pool(name="work", bufs=2))

        ident = const.tile([128, 128], F32)
        make_identity(nc, ident)

        def tp(ps_out, in_sb):
            p = in_sb.shape[0]
            nc.tensor.transpose(ps_out, in_sb, ident[:p, :p])

        # kv/score tiles straight from DRAM (rows of kvst)
        kvt = []
        for r4 in range(4):
            t = const.tile([128, 264], F32, tag=f"kv{r4}", name=f"kv{r4}")
            nc.sync.dma_start(out=t, in_=kvst[r4 * 128:(r4 + 1) * 128, :])
            kvt.append(t)
        kv_old, kv_new, sc_old, sc_new = kvt
        wts_sb = const.tile([64, 264], F32)
        nc.sync.dma_start(out=wts_sb, in_=kvst[512:576, :])

        with tc.tile_pool(name="tpsB", bufs=2, space="PSUM") as tpsB:
            # wts -> [t, h]; emit |w|*scale and sign
            for s in range(2):
                ps = tpsB.tile([128, 64], F32, tag="wtp")
                tp(ps, wts_sb[:, 4 + 132 * s:132 + 132 * s])
                wa = work.tile([128, 64], F32, tag="wa")
                nc.scalar.activation(wa, ps, mybir.ActivationFunctionType.Abs,
                                     scale=WTS_SCALE)
                nc.sync.dma_start(out=wabs[128 * s:128 * (s + 1), :], in_=wa)
                sg = work.tile([128, 64], F32, tag="sg")
                nc.scalar.activation(sg, ps,
                                     mybir.ActivationFunctionType.Sign)
                nc.sync.dma_start(out=wsgn[128 * s:128 * (s + 1), :], in_=sg)

            # ape transposed + replicated [128, 32, 8]
            ape_st = work.tile([8, D], F32, tag="ape_st")
            nc.sync.dma_start(out=ape_st, in_=ape[:])
            aps = tpsB.tile([128, 8], F32, tag="apetp")
            tp(aps, ape_st)
            apeT = const.tile([128, 8], F32)
            nc.scalar.copy(apeT, aps)
            ape_rep = const.tile([128, 32, 8], F32)
            for g in range(32):
                nc.vector.tensor_copy(ape_rep[:, g, :], apeT)

            rms_rep = const.tile([32, D], F32)
            nc.sync.dma_start(out=rms_rep, in_=bass.AP(
                tensor=rmsw, offset=0, ap=[[0, 32], [1, D]]))

            cs_st = []
            for s in range(2):
                cst = const.tile([32, D], F32, tag=f"cs{s}", name=f"cs{s}")
                nc.sync.dma_start(out=cst, in_=cs_k[32 * s:32 * s + 32, :])
                cs_st.append(cst)

            hf = []
            for s in range(2):
                h = const.tile([128, 1], F32, tag=f"hf{s}")
                nc.sync.dma_start(out=h, in_=bass.AP(
                    tensor=haloflag, offset=s, ap=[[0, 128], [1, 1]]))
                hf.append(h)

            for s in range(2):
                o = 132 * s
                gates = work.tile([128, 32, 8], F32, tag="gates")
                so_v = sc_old[:, o:o + 128].rearrange("p (g x) -> p g x", x=4)
                sn_v = sc_new[:, o + 4:o + 132].rearrange(
                    "p (g x) -> p g x", x=4)
                ko_v = kv_old[:, o:o + 128].rearrange("p (g x) -> p g x", x=4)
                kn_v = kv_new[:, o + 4:o + 132].rearrange(
                    "p (g x) -> p g x", x=4)
                nc.vector.tensor_add(gates[:, :, 0:4], so_v,
                                     ape_rep[:, :, 0:4])
                nc.vector.tensor_add(gates[:, :, 4:8], sn_v,
                                     ape_rep[:, :, 4:8])
                nc.vector.tensor_scalar(gates[:, 0, 0:4], gates[:, 0, 0:4],
                                        hf[s], None, op0=mybir.AluOpType.add)
                gmax = work.tile([128, 32], F32, tag="gmax")
                nc.vector.reduce_max(gmax, gates, axis=mybir.AxisListType.X)
                nc.vector.tensor_sub(gates, gates,
                                     gmax.to_broadcast([128, 32, 8]))
                ex = work.tile([128, 32, 8], F32, tag="ex")
                nc.scalar.activation(ex, gates,
                                     mybir.ActivationFunctionType.Exp)
                den = work.tile([128, 32], F32, tag="den")
                nc.vector.reduce_sum(den, ex, axis=mybir.AxisListType.X)
                rec = work.tile([128, 32], F32, tag="rec")
                nc.vector.reciprocal(rec, den)
                w8 = work.tile([128, 32, 8], F32, tag="w8")
                nc.vector.tensor_mul(w8, ex, rec.to_broadcast([128, 32, 8]))
                prod = work.tile([128, 32, 8], F32, tag="prod")
                nc.vector.tensor_mul(prod[:, :, 0:4], w8[:, :, 0:4], ko_v)
                nc.vector.tensor_mul(prod[:, :, 4:8], w8[:, :, 4:8], kn_v)
                comp = work.tile([128, 32], F32, tag="comp")
                nc.vector.reduce_sum(comp, prod, axis=mybir.AxisListType.X)

                cps = tpsB.tile([32, 128], F32, tag="ctp")
                tp(cps, comp)
                compT = work.tile([32, D], F32, tag="compT")
                nc.scalar.copy(compT, cps)

                sq = work.tile([32, D], F32, tag="sq")
                nc.vector.tensor_mul(sq, compT, compT)
                ssum = work.tile([32, 1], F32, tag="ssum")
                nc.vector.reduce_sum(ssum, sq, axis=mybir.AxisListType.X)
                nc.vector.tensor_scalar(ssum, ssum, 1.0 / D, EPS,
                                        op0=mybir.AluOpType.mult,
                                        op1=mybir.AluOpType.add)
                rt = work.tile([32, 1], F32, tag="rt")
                nc.scalar.sqrt(rt, ssum)
                rs = work.tile([32, 1], F32, tag="rs")
                nc.vector.reciprocal(rs, rt)
                nc.vector.tensor_scalar(compT, compT, rs, None,
                                        op0=mybir.AluOpType.mult)
                nc.vector.tensor_mul(compT, compT, rms_rep)

                co = cs_st[s][:, 0:64]
                si = cs_st[s][:, 64:128]
                x1 = compT[:, 0:64]
                x2 = compT[:, 64:128]
                tmp = work.tile([32, D], F32, tag="ktmp")
                kx = work.tile([32, D], F32, tag="kx")
                nc.vector.tensor_mul(kx[:, 0:64], x1, co)
                nc.vector.tensor_mul(tmp[:, 0:64], x2, si)
                nc.vector.tensor_sub(kx[:, 0:64], kx[:, 0:64], tmp[:, 0:64])
                nc.vector.tensor_mul(kx[:, 64:128], x2, co)
                nc.vector.tensor_mul(tmp[:, 64:128], x1, si)
                nc.vector.tensor_add(kx[:, 64:128], kx[:, 64:128],
                                     tmp[:, 64:128])
                nc.sync.dma_start(out=k_loc[32 * s:32 * s + 32, :], in_=kx)

    nc.finalize()
    return nc


# --------------------------------------------------------------------------
# LC: qk + relu*wts head-sum (ACT+GpSimd) + causal mask + top-k (DVE)
# --------------------------------------------------------------------------
def _build_lc():
    nc = bacc.Bacc()
    qro = nc.declare_dram_parameter("qro", [H, 128, 256], F32, isOutput=False)
    kt = nc.declare_dram_parameter("kt", [128, C], F32, isOutput=False)
    wabs = nc.declare_dram_parameter("wabs", [256, H], F32, isOutput=False)
    wsgn = nc.declare_dram_parameter("wsgn", [256, H], F32, isOutput=False)
    posm3 = nc.declare_dram_parameter("posm3", [256], F32, isOutput=False)
    out_idx = nc.declare_dram_parameter("out_idx", [256, TOPK], I32,
                                        isOutput=True)

    with tile.TileContext(nc) as tc, ExitStack() as ctx:
        const = ctx.enter_context(tc.tile_pool(name="const", bufs=1))
        work = ctx.enter_context(tc.tile_pool(name="work", bufs=2))
        tk = ctx.enter_context(tc.tile_pool(name="tk", bufs=2))
        yp = ctx.enter_context(tc.tile_pool(name="yp", bufs=3))
        qkps = ctx.enter_context(tc.tile_pool(name="qkps", bufs=3,
                                              space="PSUM"))

        kt_sb = const.tile([128, C], F32)
        nc.sync.dma_start(out=kt_sb, in_=kt[:])
        qrop = ctx.enter_context(tc.tile_pool(name="qrop", bufs=1))

        wabs_sb, wsgn_sb, pos_sb = [], [], []
        for tt in range(2):
            wa = const.tile([128, H], F32, tag=f"wa{tt}", name=f"wa{tt}")
            nc.sync.dma_start(out=wa, in_=wabs[tt * 128:(tt + 1) * 128, :])
            wabs_sb.append(wa)
            sg = const.tile([128, H], F32, tag=f"sg{tt}", name=f"sg{tt}")
            nc.sync.dma_start(out=sg, in_=wsgn[tt * 128:(tt + 1) * 128, :])
            wsgn_sb.append(sg)
            p = const.tile([128, 1], F32, tag=f"pos{tt}", name=f"pos{tt}")
            nc.sync.dma_start(out=p, in_=posm3[tt * 128:(tt + 1) * 128])
            pos_sb.append(p)

        c4p = const.tile([128, C], F32)
        nc.gpsimd.iota(c4p, pattern=[[4, C]], base=0, channel_multiplier=0,
                       allow_small_or_imprecise_dtypes=True)
        c4f = const.tile([128, C], F32)
        nc.vector.tensor_scalar(c4f, c4p, -1.0, None,
                                op0=mybir.AluOpType.mult)
        negs = const.tile([128, C], F32)
        nc.vector.memset(negs, NEG)
        neg1 = const.tile([128, TOPK], I32)
        nc.vector.memset(neg1, -1)

        acc = [const.tile([128, WIDTHS[tt]], F32, tag=f"acc{tt}",
                          name=f"acc{tt}") for tt in range(2)]

        def emit_topk(tt):
            W = WIDTHS[tt]
            cmp = work.tile([128, C], F32, tag="cmp", name="cmp")
            nc.vector.tensor_scalar(cmp[:, :W], c4f[:, :W], pos_sb[tt], None,
                                    op0=mybir.AluOpType.add)
            mbit = work.tile([128, C], U32, tag="mbit", name="mbit")
            nc.vector.tensor_scalar(mbit[:, :W], cmp[:, :W], 0.0, None,
                                    op0=mybir.AluOpType.is_lt)
            nc.vector.copy_predicated(acc[tt], mbit[:, :W], negs[:, :W])

            idx = tk.tile([128, TOPK], U32, tag="idx", name="idx")
            vals = acc[tt]
            for it in range(32):
                mx = tk.tile([128, 8], F32, tag="mx", name="mx")
                nc.vector.max(out=mx, in_=vals)
                nc.vector.max_index(out=idx[:, it * 8:(it + 1) * 8],
                                    in_max=mx, in_values=vals)
                nc.vector.match_replace(out=vals, in_to_replace=mx,
                                        in_values=vals, imm_value=NEG)

            idx32 = tk.tile([128, TOPK], I32, tag="idx32", name="idx32")
            nc.vector.tensor_copy(idx32, idx)
            rmp = work.tile([128, TOPK], F32, tag="rmp", name="rmp")
            nc.vector.tensor_scalar(rmp, c4f[:, :TOPK], pos_sb[tt], None,
                                    op0=mybir.AluOpType.add)
            rbit = work.tile([128, TOPK], U32, tag="rbit", name="rbit")
            nc.vector.tensor_scalar(rbit, rmp, 0.0, None,
                                    op0=mybir.AluOpType.is_lt)
            nc.vector.copy_predicated(idx32, rbit, neg1)
            nc.sync.dma_start(out=out_idx[tt * 128:(tt + 1) * 128, :],
                              in_=idx32)

        # per-head q tiles resident (loaded once, used by both tt passes)
        qts = []
        for h in range(H):
            qt = qrop.tile([128, 256], F32, tag=f"qro{h}", name=f"qro{h}")
            nc.sync.dma_start(out=qt, in_=qro[h])
            qts.append(qt)
        for tt in range(2):
            W = WIDTHS[tt]
            nc.vector.memset(acc[tt], 0.0)
            for h in range(H):
                ps_qk = qkps.tile([128, 512], F32, tag="qk", name="ps_qk")
                nc.tensor.matmul(ps_qk[:, :W],
                                 qts[h][:, tt * 128:(tt + 1) * 128],
                                 kt_sb[:, :W], start=True, stop=True)
                y = yp.tile([128, 512], F32, tag="y")
                nc.scalar.activation(y[:, :W], ps_qk[:, :W],
                                     mybir.ActivationFunctionType.Relu,
                                     scale=wabs_sb[tt][:, h:h + 1])
                nc.vector.scalar_tensor_tensor(
                    out=acc[tt], in0=y[:, :W], scalar=wsgn_sb[tt][:, h:h + 1],
                    in1=acc[tt], op0=mybir.AluOpType.mult,
                    op1=mybir.AluOpType.add)
            emit_topk(tt)

    nc.finalize()
    return nc


def _get(name):
    if name not in _cache:
        _cache[name] = {"la": _build_la, "lb": _build_lb,
                        "lc": _build_lc}[name]()
    return _cache[name]


def kernel(hidden_states, qr, positions, W_fused, Wq, Wproj, ape, rms_weight,
           cos_sin_cache, _timing=None):
    hidden_states = np.asarray(hidden_states, np.float32)
    qr = np.asarray(qr, np.float32)
    positions = np.asarray(positions, np.int32)
    W_fused = np.asarray(W_fused, np.float32)
    Wq = np.asarray(Wq, np.float32)
    Wproj = np.asarray(Wproj, np.float32)
    ape = np.asarray(ape, np.float32)
    rms_weight = np.asarray(rms_weight, np.float32)
    cos_sin_cache = np.asarray(cos_sin_cache, np.float32)

    cores = list(range(NC))
    trace = _timing is not None

    # host-side data prep (layout + fp16 hi/lo split; lo scaled by 2^11)
    def _split16(x):
        xh = x.astype(np.float16)
        xl = ((x - xh.astype(np.float32)) * 2048.0).astype(np.float16)
        return np.ascontiguousarray(xh), np.ascontiguousarray(xl)

    hidT_h, hidT_l = _split16(np.ascontiguousarray(hidden_states.T))
    wcomb_h, wcomb_l = _split16(np.ascontiguousarray(
        np.concatenate([W_fused.T, Wproj.T], axis=1)))           # [7168, 576]
    qrT_h, qrT_l = _split16(np.ascontiguousarray(qr.T))          # [1536, T]
    wq_h, wq_l = _split16(np.ascontiguousarray(
        Wq.reshape(H, 128, 12, 128).transpose(0, 3, 2, 1)))      # [h,kk,kc,m]
    csT = np.ascontiguousarray(cos_sin_cache.T)                  # [128, T]
    cc_host = np.ascontiguousarray(np.concatenate([csT[0:64], csT[0:64]]))
    ss_host = np.ascontiguousarray(
        np.concatenate([-csT[64:128], csT[64:128]]))
    perm_host = np.zeros((128, 128), np.float32)
    perm_host[np.arange(128), (np.arange(128) + 64) % 128] = 1.0

    in_la = []
    for i in cores:
        in_la.append({
            "hidt_h": np.ascontiguousarray(hidT_h[KSH * i:KSH * (i + 1)]),
            "hidt_l": np.ascontiguousarray(hidT_l[KSH * i:KSH * (i + 1)]),
            "wcomb_h": np.ascontiguousarray(wcomb_h[KSH * i:KSH * (i + 1)]),
            "wcomb_l": np.ascontiguousarray(wcomb_l[KSH * i:KSH * (i + 1)]),
            "qrt_h": qrT_h, "qrt_l": qrT_l,
            "wq_h": np.ascontiguousarray(wq_h[HSH * i:HSH * (i + 1)]),
            "wq_l": np.ascontiguousarray(wq_l[HSH * i:HSH * (i + 1)]),
            "cc": cc_host, "ss": ss_host, "perm": perm_host,
        })
    r_la = run_bass_kernel_spmd(_get("la"), in_la, cores, trace=trace,
                                trace_cores=cores if trace else None)

    kv_full = np.zeros((576, T), np.float32)
    for i in cores:
        kv_full += r_la.results[i]["kv_part"]
    qT_heads = np.empty((H, 128, T), np.float32)
    for i in cores:
        qT_heads[HSH * i:HSH * (i + 1)] = r_la.results[i]["qro"]

    in_lb = []
    for i in cores:
        kvst = np.zeros((576, 264), np.float32)
        for s in range(2):
            t0 = 256 * i + 128 * s
            lo = t0 - 4
            if lo < 0:
                kvst[:, 132 * s + 4:132 * s + 132] = kv_full[:, 0:t0 + 128]
            else:
                kvst[:, 132 * s:132 * s + 132] = kv_full[:, lo:t0 + 128]
        in_lb.append({
            "kvst": kvst, "ape": ape, "rmsw": rms_weight,
            "cs_k": np.ascontiguousarray(cos_sin_cache[64 * i:64 * i + 64]),
            "haloflag": np.array([NEG if i == 0 else 0.0, 0.0], np.float32),
        })
    r_lb = run_bass_kernel_spmd(_get("lb"), in_lb, cores, trace=trace,
                                trace_cores=cores if trace else None)

    k_full = np.concatenate([r_lb.results[i]["k_loc"] for i in cores], axis=0)
    kT_host = np.ascontiguousarray(k_full.T)                     # [128, C]
    wabs_full = np.concatenate([r_lb.results[i]["wabs"] for i in cores])
    wsgn_full = np.concatenate([r_lb.results[i]["wsgn"] for i in cores])

    in_lc = []
    sels = []
    for i in cores:
        sel = np.concatenate([np.arange(128 * i, 128 * i + 128),
                              np.arange(128 * (15 - i), 128 * (15 - i) + 128)])
        sels.append(sel)
        in_lc.append({
            "qro": np.ascontiguousarray(qT_heads[:, :, sel]),
            "kt": kT_host,
            "wabs": np.ascontiguousarray(wabs_full[sel]),
            "wsgn": np.ascontiguousarray(wsgn_full[sel]),
            "posm3": (positions[sel] - 3).astype(np.float32),
        })
    r_lc = run_bass_kernel_spmd(_get("lc"), in_lc, cores, trace=trace,
                                trace_cores=cores if trace else None)

    out = np.empty((T, TOPK), np.int32)
    for i in cores:
        out[sels[i]] = r_lc.results[i]["out_idx"]

    if _timing is not None:
        _timing["la"] = r_la
        _timing["lb"] = r_lb
        _timing["lc"] = r_lc
    return out
